# revision 24
# baseline (speedup 1.0000x reference)
"""Bass/Trainium2 kernel for nn_CrossWindowAttention3D (8-core SPMD).

Strategy (hardcoded for shapes B_=1024, N=98, C=96, H=3, NW=512):
- Shard 1024 window-instances over 8 cores: core c owns distinct windows
  [64c, 64c+64) for both batch replicas, interleaved (b0,j),(b1,j) so the
  mask+bias table for window j is loaded once per pair.
- Host precomputes channel-major bf16 transposes of x/y, per-head composite
  matrices M_h = scale * Wq_h^T Wk_h (so logits_h = x^T (M_h y) and no
  separate k projection is needed), the additive mask+bias table
  amb[tk, j, h*98+tq] = mask[j, tq, tk] + bias[h, tq, tk], and the proj
  weight with a trailing bias row (pw2 [97, 96], row 96 = proj_b).
- Device, per 4-window group (2 pairs x 2 batch replicas):
  * zt = M_h y for 4 windows (3 full matmuls, 392 cols each)
  * per window: mask+bias accumulated into logits PSUM via an
    identity-stationary matmul, then logits = x^T zt on top (PSUM acc);
    the same x stationary also produces token-major v (v = x^T Wv).
  * one quad-window exp on ACT ([98, 4, 294] spanning 4 PSUM banks)
  * softmax denominators via narrow ones-stationary matmuls (out 32-row
    col-groups run concurrently in the PE array), reciprocal via the
    1-op DVE reciprocal_approx_fast, av via 12 narrow matmuls, normalize
    with one DVE multiply into a [97, 392] tile whose last row is 1.0,
    projection (+bias via the ones row) as one 392-col matmul, output
    staged bf16 and DMA'd channel-major; host transposes/casts to f32.
- PSUM budget: shared 4-bank region (zt slots then logits), pv 1, pd 1,
  pav 1, psy 1 = 8 banks. Emission is software-pipelined: group g+1's zt
  and logits interleave with group g's denominator/av/proj tail.
"""

import sys

sys.path.insert(0, "/opt/trn_rl_repo")

import numpy as np
import ml_dtypes

import concourse.bass as bass
import concourse.tile as tile
from concourse import mybir
from concourse.vector_clock import ScopedClock
from concourse.bass_utils import run_bass_kernel_spmd

BF16 = mybir.dt.bfloat16
F32 = mybir.dt.float32
NPBF16 = ml_dtypes.bfloat16

WS = (2, 7, 7)
N = 98            # tokens per window
C = 96            # embed dim
H = 3             # heads
HD = 32           # head dim
NW = 512          # distinct windows
BWIN = 1024       # window-instances total
NCORES = 8
NI = 128          # instances per core
NJ = 64           # distinct windows per core
T = NI * N        # tokens per core = 12544
HB = H * N        # 294
G = 4             # windows per group
NG = NI // G      # 32 groups


# ---------------------------------------------------------------- tile patch
def _patch_tile_tail_drain():
    """This neuronxcc build rejects >1 sync wait on CTRL-class (Drain)
    instructions; split the TileContext tail-drain waits across NOPs."""
    if getattr(tile.TileContext, "_drain_patch_applied", False):
        return

    def _drain_and_barrier_split(self, tick_clock, wait_clock):
        nc = self.nc
        carrier = nc.sync.nop(nofuse=True)
        wait_clock.add_sem_waits(
            carrier.ins, ScopedClock({None: tick_clock.global_clock})
        )
        si = carrier.ins.sync_info
        waits = list(si.on_wait or []) if si is not None else []
        if len(waits) > 1:
            si.on_wait = waits[:1]
            for w in waits[1:]:
                extra = nc.sync.nop(nofuse=True)
                esi = extra.ins.sync_info
                if esi is None:
                    extra.ins.sync_info = mybir.SyncInfo(
                        on_wait=[w], on_update=[]
                    )
                else:
                    esi.on_wait = list(esi.on_wait or []) + [w]
        nc.sync.drain()
        nc.all_engine_barrier()
        assert self.sems is not None
        popped = nc._tile_sem_poison_stack.pop()
        assert popped is self._sem_poison
        nc.clear_and_free_semaphores(list(self.sems.allocated().values()))
        nc.all_engine_barrier()

    tile.TileContext._drain_and_barrier = _drain_and_barrier_split
    tile.TileContext._drain_patch_applied = True


def _split_sync_waits(nc, max_waits=1):
    """This neuronxcc build accepts at most one sync wait per instruction.
    Hoist excess waits onto same-engine NOPs inserted just before the
    instruction (the sequencer blocks on them in order; AND-semantics of
    multiple waits is preserved)."""
    ctr = 0
    for bb in nc.main_func.blocks:
        new_list = []
        changed = False
        for inst in bb.instructions:
            si = inst.sync_info
            waits = list(si.on_wait or []) if si is not None else []
            if len(waits) > max_waits:
                si.on_wait = waits[: max_waits]
                for w in waits[max_waits:]:
                    nop = mybir.InstNoOp(
                        name=f"I-waitsplit-{ctr}", ins=[], outs=[]
                    )
                    ctr += 1
                    nop.engine = inst.engine
                    nop.sync_info = mybir.SyncInfo(on_wait=[w], on_update=[])
                    new_list.append(nop)
                changed = True
            new_list.append(inst)
        if changed:
            bb.instructions = new_list


# ------------------------------------------------------------- host helpers
def _relative_position_index():
    ws = WS
    coords = np.stack(
        np.meshgrid(
            np.arange(ws[0]), np.arange(ws[1]), np.arange(ws[2]), indexing="ij"
        )
    )
    cf = coords.reshape(3, -1)
    rel = cf[:, :, None] - cf[:, None, :]
    rel = rel.transpose(1, 2, 0).astype(np.int64)
    rel[..., 0] += ws[0] - 1
    rel[..., 1] += ws[1] - 1
    rel[..., 2] += ws[2] - 1
    rel[..., 0] *= (2 * ws[1] - 1) * (2 * ws[2] - 1)
    rel[..., 1] *= 2 * ws[2] - 1
    return rel.sum(-1)  # (N, N)


REL_IDX = _relative_position_index()


# ------------------------------------------------------------ device program
_PROGRAM = None

# tiling knobs
XCH = 32          # instances per x/y SBUF chunk (4 chunks)
ACH = 16          # distinct windows per amb SBUF chunk
YB = 8            # windows per output staging buffer / DMA


def _build_program(split_waits=True):
    _patch_tile_tail_drain()
    nc = bass.Bass()

    xT = nc.declare_dram_parameter("xT", [C, T], BF16, isOutput=False)
    yT = nc.declare_dram_parameter("yT", [C, T], BF16, isOutput=False)
    emb = nc.declare_dram_parameter("emb", [N, NJ, HB], BF16, isOutput=False)
    zw = nc.declare_dram_parameter("zw", [C, H, C], BF16, isOutput=False)
    wv = nc.declare_dram_parameter("wv", [C, C], BF16, isOutput=False)
    pw2 = nc.declare_dram_parameter("pw2", [C + 1, C], BF16, isOutput=False)
    out = nc.declare_dram_parameter("yT_out", [C, T], BF16, isOutput=True)

    from contextlib import ExitStack

    with tile.TileContext(nc) as tc:
        with ExitStack() as ctx:
            singles = ctx.enter_context(tc.tile_pool(name="singles", bufs=1))
            xt_pool = ctx.enter_context(tc.tile_pool(name="xt", bufs=2))
            yt_pool = ctx.enter_context(tc.tile_pool(name="yt", bufs=2))
            amb_pool = ctx.enter_context(tc.tile_pool(name="amb", bufs=2))
            ztq_pool = ctx.enter_context(tc.tile_pool(name="ztq", bufs=2))
            expU_pool = ctx.enter_context(tc.tile_pool(name="expU", bufs=2))
            expT_pool = ctx.enter_context(tc.tile_pool(name="expT", bufs=2))
            v4_pool = ctx.enter_context(tc.tile_pool(name="v4", bufs=2))
            r2_pool = ctx.enter_context(tc.tile_pool(name="r2", bufs=2))
            avT_pool = ctx.enter_context(tc.tile_pool(name="avT", bufs=2))
            ystage_pool = ctx.enter_context(
                tc.tile_pool(name="ystage", bufs=2)
            )
            # PSUM (8 banks): per-pair logits tiles [98,2,512] bufs=2 (4),
            # zt [96,3,512] (3), and ONE bank time-shared by pv/pd/psy via
            # a common tag ring (their lifetimes are strictly ordered).
            ps_lg = ctx.enter_context(
                tc.tile_pool(name="ps_lg", bufs=2, space="PSUM")
            )
            ps_zt = ctx.enter_context(
                tc.tile_pool(name="ps_zt", bufs=1, space="PSUM")
            )
            ps_aux = ctx.enter_context(
                tc.tile_pool(name="ps_aux", bufs=1, space="PSUM")
            )

            zw_sb = singles.tile([C, H, C], BF16)
            nc.sync.dma_start(out=zw_sb, in_=zw[:, :, :])
            wv_sb = singles.tile([C, C], BF16)
            nc.sync.dma_start(out=wv_sb, in_=wv[:, :])
            pw2_sb = singles.tile([C + 1, C], BF16)
            nc.sync.dma_start(out=pw2_sb, in_=pw2[:, :])
            ones_sb = singles.tile([N, HD], BF16)
            nc.vector.memset(ones_sb, 1.0)

            # per-group state carried across the pipelined loop
            st = {}
            xt_ch = yt_ch = amb_ch = None
            ystage = None

            def emit_head_a(g):
                """chunk loads + zt matmuls for heads 0..1 of group g."""
                nonlocal xt_ch, yt_ch, amb_ch
                w0 = G * g
                if w0 % XCH == 0:
                    ch = w0 // XCH
                    xt_ch = xt_pool.tile([C, XCH * N], BF16)
                    nc.sync.dma_start(
                        out=xt_ch,
                        in_=xT[:, ch * XCH * N : (ch + 1) * XCH * N],
                    )
                    yt_ch = yt_pool.tile([C, XCH * N], BF16)
                    nc.sync.dma_start(
                        out=yt_ch,
                        in_=yT[:, ch * XCH * N : (ch + 1) * XCH * N],
                    )
                # distinct windows for group g are 2g, 2g+1
                if (2 * g) % ACH == 0:
                    ak = (2 * g) // ACH
                    amb_ch = amb_pool.tile([N, ACH, HB], BF16)
                    nc.sync.dma_start(
                        out=amb_ch, in_=emb[:, ak * ACH : (ak + 1) * ACH, :]
                    )

                pz = ps_zt.tile([C, H, 512], F32)
                goff = (w0 % XCH) * N
                for h in range(2):
                    nc.tensor.matmul(
                        out=pz[:, h, 0 : G * N],
                        lhsT=zw_sb[:, h, :],
                        rhs=yt_ch[:, goff : goff + G * N],
                    )
                st[g] = {
                    "pz": pz,
                    "goff": goff,
                    "amb_ch": amb_ch,
                    "xt_ch": xt_ch,
                    "yt_ch": yt_ch,
                }

            def emit_head_b(g):
                """zt matmul head 2 + PSUM->SBUF cast for group g."""
                sg = st[g]
                pz = sg.pop("pz")
                goff = sg["goff"]
                nc.tensor.matmul(
                    out=pz[:, 2, 0 : G * N],
                    lhsT=zw_sb[:, 2, :],
                    rhs=sg.pop("yt_ch")[:, goff : goff + G * N],
                )
                ztq = ztq_pool.tile([C, H, G * N], BF16)
                nc.vector.tensor_copy(ztq, pz[:, :, 0 : G * N])
                sg["ztq"] = ztq

            def emit_logits(g):
                """mask+bias accumulate, logits, v, exp, vcopy for group g."""
                sg = st[g]
                ztq = sg["ztq"]
                goff = sg["goff"]
                a_ch = sg["amb_ch"]
                x_ch = sg["xt_ch"]
                pv = ps_aux.tile([N, G, 128], F32, name="pv", tag="aux")
                expU = expU_pool.tile([N, G, HB], BF16)
                expT = expT_pool.tile([N, G, HB], BF16)
                Rp0 = None
                for pr in range(2):
                    j = 2 * g + pr           # distinct window
                    aj = j % ACH
                    Rp = ps_lg.tile([N, 2, 512], F32, name="Rp")
                    if pr == 0:
                        Rp0 = Rp
                    for k in range(2):
                        w = 2 * pr + k       # window slot in group
                        col = goff + w * N
                        nc.tensor.matmul(
                            out=Rp[:, k, 0:HB],
                            lhsT=x_ch[:, col : col + N],
                            rhs=ztq[:, :, w * N : (w + 1) * N],
                        )
                    nc.scalar.activation(
                        out=expU[:, 2 * pr : 2 * pr + 2, :],
                        in_=Rp[:, :, 0:HB],
                        func=mybir.ActivationFunctionType.Exp,
                    )
                    # mask+bias applied multiplicatively on the idle Pool
                    # engine: expT = exp(logits) * exp(mask+bias)
                    emb_b = a_ch[:, aj : aj + 1, :].broadcast_to((N, 2, HB))
                    nc.gpsimd.tensor_tensor(
                        out=expT[:, 2 * pr : 2 * pr + 2, :],
                        in0=expU[:, 2 * pr : 2 * pr + 2, :],
                        in1=emb_b,
                        op=mybir.AluOpType.mult,
                    )
                for w in range(G):
                    col = goff + w * N
                    nc.tensor.matmul(
                        out=pv[:, w, 0:C],
                        lhsT=x_ch[:, col : col + N],
                        rhs=wv_sb,
                    )
                v4 = v4_pool.tile([N, G, C], BF16)
                nc.vector.tensor_copy(v4, pv[:, :, 0:C])
                sg["expT"] = expT
                sg["v4"] = v4
                sg["Rp0"] = Rp0

            def emit_tail_a(g):
                """denominators + Ln for group g."""
                sg = st[g]
                expT = sg["expT"]
                pd = ps_aux.tile([C, G, N], F32, name="pd", tag="aux")
                for h in range(H):
                    nc.tensor.matmul(
                        out=pd[h * HD : (h + 1) * HD, :, :],
                        lhsT=ones_sb,
                        rhs=expT[:, :, h * N : (h + 1) * N],
                    )
                # 1/d = exp(-ln(d)); Ln+Exp share one ACT table set
                t_ln = r2_pool.tile([C, G, N], F32, name="t_ln", tag="tl")
                nc.scalar.activation(
                    out=t_ln,
                    in_=pd,
                    func=mybir.ActivationFunctionType.Ln,
                )
                sg["t_ln"] = t_ln

            def emit_tail_b1(g):
                """av matmuls, reciprocal finish, normalize for group g."""
                sg = st[g]
                expT = sg["expT"]
                v4 = sg["v4"]
                pav = ps_aux.tile([C, G, 128], F32, name="pav", tag="aux")
                for w in range(G):
                    for h in range(H):
                        nc.tensor.matmul(
                            out=pav[h * HD : (h + 1) * HD, w, 0:N],
                            lhsT=v4[:, w, h * HD : (h + 1) * HD],
                            rhs=expT[:, w, h * N : (h + 1) * N],
                        )
                r2 = r2_pool.tile([C, G, N], F32, name="r2", tag="r2")
                nc.scalar.activation(
                    out=r2,
                    in_=sg.pop("t_ln"),
                    func=mybir.ActivationFunctionType.Exp,
                    scale=-1.0,
                )
                avT = avT_pool.tile([C + 1, G, N], BF16)
                if g < 2:
                    nc.gpsimd.memset(avT[C : C + 1, :, :], 1.0)
                nc.vector.tensor_tensor(
                    out=avT[0:C, :, :],
                    in0=pav[:, :, 0:N],
                    in1=r2,
                    op=mybir.AluOpType.mult,
                )
                sg["avT"] = avT

            def emit_tail_b2(g):
                """projection + output staging + DMA for group g."""
                nonlocal ystage
                sg = st.pop(g)
                psy = ps_aux.tile([C, G * N], F32, name="psy", tag="aux")
                nc.tensor.matmul(out=psy, lhsT=pw2_sb, rhs=sg["avT"])
                if g % 2 == 0:
                    ystage = ystage_pool.tile([C, 2, G * N], BF16)
                nc.vector.tensor_copy(ystage[:, g % 2, :], psy)
                if g % 2 == 1:
                    blk = g // 2
                    nc.sync.dma_start(
                        out=out[:, blk * YB * N : (blk + 1) * YB * N],
                        in_=ystage,
                    )

            # Software pipeline: group g's tail work runs one full
            # iteration after its logits, so every tail dependency
            # (exp -> Pool mult -> denom, Ln -> Exp -> avT) has a whole
            # group of slack and the tensor queue never stalls on it.
            emit_head_a(0)
            emit_head_b(0)
            for it in range(NG):
                emit_logits(it)
                if it > 0:
                    emit_tail_a(it - 1)
                if it + 1 < NG:
                    emit_head_a(it + 1)
                if it > 0:
                    emit_tail_b1(it - 1)
                if it + 1 < NG:
                    emit_head_b(it + 1)
                if it > 0:
                    emit_tail_b2(it - 1)
            emit_tail_a(NG - 1)
            emit_tail_b1(NG - 1)
            emit_tail_b2(NG - 1)

    if split_waits:
        _split_sync_waits(nc)
    return nc


def _get_program():
    global _PROGRAM
    if _PROGRAM is None:
        _PROGRAM = _build_program()
    return _PROGRAM


# ------------------------------------------------------------------- kernel
def _core_instance_bidx(c):
    """B_ indices for core c's 128 window-instances, in device order."""
    w = np.arange(NI)
    return 512 * (w % 2) + NJ * c + (w // 2)


def _prepare_in_maps(x, y, mask, qkv_w, rpb_table, proj_w, proj_b):
    x = np.asarray(x, dtype=np.float32)
    y = np.asarray(y, dtype=np.float32)
    mask = np.asarray(mask, dtype=np.float32)
    qkv_w = np.asarray(qkv_w, dtype=np.float64)
    rpb_table = np.asarray(rpb_table, dtype=np.float32)
    proj_w = np.asarray(proj_w, dtype=np.float32)
    proj_b = np.asarray(proj_b, dtype=np.float32)

    scale = float(HD) ** -0.5

    # multiplicative mask+bias table: emb[tk, j, h*98+tq] = exp(mask+bias)
    bias = rpb_table[REL_IDX.reshape(-1)].reshape(N, N, H).transpose(2, 0, 1)
    emb_all = np.exp(mask[:, None, :, :] + bias[None, :, :, :])
    emb_t = np.ascontiguousarray(emb_all.transpose(3, 0, 1, 2)).reshape(
        N, NW, HB
    )

    # per-head composite: zw[:, h, :] = scale * Wq_h^T @ Wk_h  (96x96)
    zw_h = np.empty((C, H, C), dtype=np.float64)
    for h in range(H):
        wq_h = qkv_w[h * HD : (h + 1) * HD, :]            # (32, 96)
        wk_h = qkv_w[C + h * HD : C + (h + 1) * HD, :]    # (32, 96)
        zw_h[:, h, :] = scale * (wq_h.T @ wk_h)
    zw_h = zw_h.astype(NPBF16)

    wv_h = np.ascontiguousarray(qkv_w[2 * C : 3 * C].astype(np.float32).T
                                ).astype(NPBF16)
    pw2_h = np.empty((C + 1, C), dtype=np.float32)
    pw2_h[0:C] = proj_w.T
    pw2_h[C] = proj_b
    pw2_h = pw2_h.astype(NPBF16)

    in_maps = []
    bidx = []
    for c in range(NCORES):
        bi = _core_instance_bidx(c)
        bidx.append(bi)
        xc = x[bi].reshape(T, C)
        yc = y[bi].reshape(T, C)
        emb_c = np.ascontiguousarray(
            emb_t[:, NJ * c : NJ * (c + 1), :]
        ).astype(NPBF16)
        in_maps.append(
            {
                "xT": np.ascontiguousarray(xc.T).astype(NPBF16),
                "yT": np.ascontiguousarray(yc.T).astype(NPBF16),
                "emb": emb_c,
                "zw": zw_h,
                "wv": wv_h,
                "pw2": pw2_h,
            }
        )
    return in_maps, bidx


def kernel(x, y, mask, qkv_w, rpb_table, proj_w, proj_b):
    in_maps, bidx = _prepare_in_maps(
        x, y, mask, qkv_w, rpb_table, proj_w, proj_b
    )
    nc = _get_program()
    res = run_bass_kernel_spmd(nc, in_maps, list(range(NCORES)))

    out_full = np.empty((BWIN, N, C), dtype=np.float32)
    for c in range(NCORES):
        yt_o = np.asarray(res.results[c]["yT_out"]).astype(np.float32)
        out_full[bidx[c]] = yt_o.T.reshape(NI, N, C)
    return out_full


# revision 26
# speedup vs baseline: 1.0112x; 1.0112x over previous
"""Bass/Trainium2 kernel for nn_CrossWindowAttention3D (8-core SPMD).

Strategy (hardcoded for shapes B_=1024, N=98, C=96, H=3, NW=512):
- Shard 1024 window-instances over 8 cores: core c owns distinct windows
  [64c, 64c+64) for both batch replicas, interleaved (b0,j),(b1,j) so the
  mask+bias table for window j is loaded once per pair.
- Host precomputes channel-major bf16 transposes of x/y, per-head composite
  matrices M_h = scale * Wq_h^T Wk_h (so logits_h = x^T (M_h y) and no
  separate k projection is needed), the additive mask+bias table
  amb[tk, j, h*98+tq] = mask[j, tq, tk] + bias[h, tq, tk], and the proj
  weight with a trailing bias row (pw2 [97, 96], row 96 = proj_b).
- Device, per 4-window group (2 pairs x 2 batch replicas):
  * zt = M_h y for 4 windows (3 full matmuls, 392 cols each)
  * per window: mask+bias accumulated into logits PSUM via an
    identity-stationary matmul, then logits = x^T zt on top (PSUM acc);
    the same x stationary also produces token-major v (v = x^T Wv).
  * one quad-window exp on ACT ([98, 4, 294] spanning 4 PSUM banks)
  * softmax denominators via narrow ones-stationary matmuls (out 32-row
    col-groups run concurrently in the PE array), reciprocal via the
    1-op DVE reciprocal_approx_fast, av via 12 narrow matmuls, normalize
    with one DVE multiply into a [97, 392] tile whose last row is 1.0,
    projection (+bias via the ones row) as one 392-col matmul, output
    staged bf16 and DMA'd channel-major; host transposes/casts to f32.
- PSUM budget: shared 4-bank region (zt slots then logits), pv 1, pd 1,
  pav 1, psy 1 = 8 banks. Emission is software-pipelined: group g+1's zt
  and logits interleave with group g's denominator/av/proj tail.
"""

import sys

sys.path.insert(0, "/opt/trn_rl_repo")

import numpy as np
import ml_dtypes

import concourse.bass as bass
import concourse.tile as tile
from concourse import mybir
from concourse.vector_clock import ScopedClock
from concourse.bass_utils import run_bass_kernel_spmd

BF16 = mybir.dt.bfloat16
F32 = mybir.dt.float32
NPBF16 = ml_dtypes.bfloat16

WS = (2, 7, 7)
N = 98            # tokens per window
C = 96            # embed dim
H = 3             # heads
HD = 32           # head dim
NW = 512          # distinct windows
BWIN = 1024       # window-instances total
NCORES = 8
NI = 128          # instances per core
NJ = 64           # distinct windows per core
T = NI * N        # tokens per core = 12544
HB = H * N        # 294
G = 4             # windows per group
NG = NI // G      # 32 groups


# ---------------------------------------------------------------- tile patch
def _patch_tile_tail_drain():
    """This neuronxcc build rejects >1 sync wait on CTRL-class (Drain)
    instructions; split the TileContext tail-drain waits across NOPs."""
    if getattr(tile.TileContext, "_drain_patch_applied", False):
        return

    def _drain_and_barrier_split(self, tick_clock, wait_clock):
        nc = self.nc
        carrier = nc.sync.nop(nofuse=True)
        wait_clock.add_sem_waits(
            carrier.ins, ScopedClock({None: tick_clock.global_clock})
        )
        si = carrier.ins.sync_info
        waits = list(si.on_wait or []) if si is not None else []
        if len(waits) > 1:
            si.on_wait = waits[:1]
            for w in waits[1:]:
                extra = nc.sync.nop(nofuse=True)
                esi = extra.ins.sync_info
                if esi is None:
                    extra.ins.sync_info = mybir.SyncInfo(
                        on_wait=[w], on_update=[]
                    )
                else:
                    esi.on_wait = list(esi.on_wait or []) + [w]
        nc.sync.drain()
        nc.all_engine_barrier()
        assert self.sems is not None
        popped = nc._tile_sem_poison_stack.pop()
        assert popped is self._sem_poison
        nc.clear_and_free_semaphores(list(self.sems.allocated().values()))
        nc.all_engine_barrier()

    tile.TileContext._drain_and_barrier = _drain_and_barrier_split
    tile.TileContext._drain_patch_applied = True


def _split_sync_waits(nc, max_waits=1):
    """This neuronxcc build accepts at most one sync wait per instruction.
    Hoist excess waits onto same-engine NOPs inserted just before the
    instruction (the sequencer blocks on them in order; AND-semantics of
    multiple waits is preserved)."""
    ctr = 0
    for bb in nc.main_func.blocks:
        new_list = []
        changed = False
        for inst in bb.instructions:
            si = inst.sync_info
            waits = list(si.on_wait or []) if si is not None else []
            if len(waits) > max_waits:
                si.on_wait = waits[: max_waits]
                for w in waits[max_waits:]:
                    nop = mybir.InstNoOp(
                        name=f"I-waitsplit-{ctr}", ins=[], outs=[]
                    )
                    ctr += 1
                    nop.engine = inst.engine
                    nop.sync_info = mybir.SyncInfo(on_wait=[w], on_update=[])
                    new_list.append(nop)
                changed = True
            new_list.append(inst)
        if changed:
            bb.instructions = new_list


# ------------------------------------------------------------- host helpers
def _relative_position_index():
    ws = WS
    coords = np.stack(
        np.meshgrid(
            np.arange(ws[0]), np.arange(ws[1]), np.arange(ws[2]), indexing="ij"
        )
    )
    cf = coords.reshape(3, -1)
    rel = cf[:, :, None] - cf[:, None, :]
    rel = rel.transpose(1, 2, 0).astype(np.int64)
    rel[..., 0] += ws[0] - 1
    rel[..., 1] += ws[1] - 1
    rel[..., 2] += ws[2] - 1
    rel[..., 0] *= (2 * ws[1] - 1) * (2 * ws[2] - 1)
    rel[..., 1] *= 2 * ws[2] - 1
    return rel.sum(-1)  # (N, N)


REL_IDX = _relative_position_index()


# ------------------------------------------------------------ device program
_PROGRAM = None

# tiling knobs
XCH = 32          # instances per x/y SBUF chunk (4 chunks)
ACH = 16          # distinct windows per amb SBUF chunk
YB = 8            # windows per output staging buffer / DMA


def _build_program(split_waits=True):
    _patch_tile_tail_drain()
    nc = bass.Bass()

    xT = nc.declare_dram_parameter("xT", [C, T], BF16, isOutput=False)
    yT = nc.declare_dram_parameter("yT", [C, T], BF16, isOutput=False)
    emb = nc.declare_dram_parameter("emb", [N, NJ, HB], BF16, isOutput=False)
    zw = nc.declare_dram_parameter("zw", [C, H, C], BF16, isOutput=False)
    wv = nc.declare_dram_parameter("wv", [C, C], BF16, isOutput=False)
    pw2 = nc.declare_dram_parameter("pw2", [C + 1, C], BF16, isOutput=False)
    out = nc.declare_dram_parameter("yT_out", [C, T], BF16, isOutput=True)

    from contextlib import ExitStack

    with tile.TileContext(nc) as tc:
        with ExitStack() as ctx:
            singles = ctx.enter_context(tc.tile_pool(name="singles", bufs=1))
            xt_pool = ctx.enter_context(tc.tile_pool(name="xt", bufs=2))
            yt_pool = ctx.enter_context(tc.tile_pool(name="yt", bufs=2))
            amb_pool = ctx.enter_context(tc.tile_pool(name="amb", bufs=2))
            ztq_pool = ctx.enter_context(tc.tile_pool(name="ztq", bufs=2))
            expU_pool = ctx.enter_context(tc.tile_pool(name="expU", bufs=2))
            expT_pool = ctx.enter_context(tc.tile_pool(name="expT", bufs=2))
            v4_pool = ctx.enter_context(tc.tile_pool(name="v4", bufs=2))
            r2_pool = ctx.enter_context(tc.tile_pool(name="r2", bufs=2))
            avT_pool = ctx.enter_context(tc.tile_pool(name="avT", bufs=2))
            ystage_pool = ctx.enter_context(
                tc.tile_pool(name="ystage", bufs=2)
            )
            # PSUM (8 banks): per-pair logits tiles [98,2,512] bufs=2 (4),
            # zt [96,3,512] (3), and ONE bank time-shared by pv/pd/psy via
            # a common tag ring (their lifetimes are strictly ordered).
            ps_lg = ctx.enter_context(
                tc.tile_pool(name="ps_lg", bufs=2, space="PSUM")
            )
            ps_zt = ctx.enter_context(
                tc.tile_pool(name="ps_zt", bufs=1, space="PSUM")
            )
            ps_aux = ctx.enter_context(
                tc.tile_pool(name="ps_aux", bufs=1, space="PSUM")
            )

            zw_sb = singles.tile([C, H, C], BF16)
            nc.sync.dma_start(out=zw_sb, in_=zw[:, :, :])
            wv_sb = singles.tile([C, C], BF16)
            nc.sync.dma_start(out=wv_sb, in_=wv[:, :])
            pw2_sb = singles.tile([C + 1, C], BF16)
            nc.sync.dma_start(out=pw2_sb, in_=pw2[:, :])
            ones_sb = singles.tile([N, HD], BF16)
            nc.vector.memset(ones_sb, 1.0)

            # per-group state carried across the pipelined loop
            st = {}
            xt_ch = yt_ch = amb_ch = None
            ystage = None

            def emit_head_a(g):
                """chunk loads + zt matmuls for heads 0..1 of group g."""
                nonlocal xt_ch, yt_ch, amb_ch
                w0 = G * g
                if w0 % XCH == 0:
                    ch = w0 // XCH
                    xt_ch = xt_pool.tile([C, XCH * N], BF16)
                    nc.sync.dma_start(
                        out=xt_ch,
                        in_=xT[:, ch * XCH * N : (ch + 1) * XCH * N],
                    )
                    yt_ch = yt_pool.tile([C, XCH * N], BF16)
                    nc.sync.dma_start(
                        out=yt_ch,
                        in_=yT[:, ch * XCH * N : (ch + 1) * XCH * N],
                    )
                # distinct windows for group g are 2g, 2g+1
                if (2 * g) % ACH == 0:
                    ak = (2 * g) // ACH
                    amb_ch = amb_pool.tile([N, ACH, HB], BF16)
                    nc.sync.dma_start(
                        out=amb_ch, in_=emb[:, ak * ACH : (ak + 1) * ACH, :]
                    )

                pz = ps_zt.tile([C, H, 512], F32)
                goff = (w0 % XCH) * N
                for h in range(2):
                    nc.tensor.matmul(
                        out=pz[:, h, 0 : G * N],
                        lhsT=zw_sb[:, h, :],
                        rhs=yt_ch[:, goff : goff + G * N],
                    )
                st[g] = {
                    "pz": pz,
                    "goff": goff,
                    "amb_ch": amb_ch,
                    "xt_ch": xt_ch,
                    "yt_ch": yt_ch,
                }

            def emit_head_b(g):
                """zt matmul head 2 + PSUM->SBUF cast for group g."""
                sg = st[g]
                pz = sg.pop("pz")
                goff = sg["goff"]
                nc.tensor.matmul(
                    out=pz[:, 2, 0 : G * N],
                    lhsT=zw_sb[:, 2, :],
                    rhs=sg.pop("yt_ch")[:, goff : goff + G * N],
                )
                ztq = ztq_pool.tile([C, H, G * N], BF16)
                nc.vector.tensor_copy(ztq, pz[:, :, 0 : G * N])
                sg["ztq"] = ztq

            def emit_logits(g):
                """mask+bias accumulate, logits, v, exp, vcopy for group g."""
                sg = st[g]
                ztq = sg["ztq"]
                goff = sg["goff"]
                a_ch = sg["amb_ch"]
                x_ch = sg["xt_ch"]
                pv = ps_aux.tile([N, G, 128], F32, name="pv", tag="aux")
                expU = expU_pool.tile([N, G, HB], BF16)
                expT = expT_pool.tile([N, G, HB], BF16)
                Rp0 = None
                for pr in range(2):
                    j = 2 * g + pr           # distinct window
                    aj = j % ACH
                    Rp = ps_lg.tile([N, 2, 512], F32, name="Rp")
                    if pr == 0:
                        Rp0 = Rp
                    for k in range(2):
                        w = 2 * pr + k       # window slot in group
                        col = goff + w * N
                        nc.tensor.matmul(
                            out=Rp[:, k, 0:HB],
                            lhsT=x_ch[:, col : col + N],
                            rhs=ztq[:, :, w * N : (w + 1) * N],
                        )
                    nc.scalar.activation(
                        out=expU[:, 2 * pr : 2 * pr + 2, :],
                        in_=Rp[:, :, 0:HB],
                        func=mybir.ActivationFunctionType.Exp,
                    )
                    # mask+bias applied multiplicatively on the idle Pool
                    # engine: expT = exp(logits) * exp(mask+bias)
                    emb_b = a_ch[:, aj : aj + 1, :].broadcast_to((N, 2, HB))
                    nc.gpsimd.tensor_tensor(
                        out=expT[:, 2 * pr : 2 * pr + 2, :],
                        in0=expU[:, 2 * pr : 2 * pr + 2, :],
                        in1=emb_b,
                        op=mybir.AluOpType.mult,
                    )
                for w in range(G):
                    col = goff + w * N
                    nc.tensor.matmul(
                        out=pv[:, w, 0:C],
                        lhsT=x_ch[:, col : col + N],
                        rhs=wv_sb,
                    )
                v4 = v4_pool.tile([N, G, C], BF16)
                nc.vector.tensor_copy(v4, pv[:, :, 0:C])
                sg["expT"] = expT
                sg["v4"] = v4
                sg["Rp0"] = Rp0

            def emit_tail_a(g):
                """denominators + Ln for group g."""
                sg = st[g]
                expT = sg["expT"]
                pd = ps_aux.tile([C, G, N], F32, name="pd", tag="aux")
                for h in range(H):
                    nc.tensor.matmul(
                        out=pd[h * HD : (h + 1) * HD, :, :],
                        lhsT=ones_sb,
                        rhs=expT[:, :, h * N : (h + 1) * N],
                    )
                # 1/d = exp(-ln(d)); Ln+Exp share one ACT table set
                t_ln = r2_pool.tile([C, G, N], F32, name="t_ln", tag="tl")
                nc.scalar.activation(
                    out=t_ln,
                    in_=pd,
                    func=mybir.ActivationFunctionType.Ln,
                )
                sg["t_ln"] = t_ln

            def emit_tail_b1(g):
                """av matmuls, reciprocal finish, normalize for group g."""
                sg = st[g]
                expT = sg["expT"]
                v4 = sg["v4"]
                pav = ps_aux.tile([C, G, 128], F32, name="pav", tag="aux")
                for w in range(G):
                    for h in range(H):
                        nc.tensor.matmul(
                            out=pav[h * HD : (h + 1) * HD, w, 0:N],
                            lhsT=v4[:, w, h * HD : (h + 1) * HD],
                            rhs=expT[:, w, h * N : (h + 1) * N],
                        )
                r2 = r2_pool.tile([C, G, N], F32, name="r2", tag="r2")
                nc.scalar.activation(
                    out=r2,
                    in_=sg.pop("t_ln"),
                    func=mybir.ActivationFunctionType.Exp,
                    scale=-1.0,
                )
                avT = avT_pool.tile([C + 1, G, N], BF16)
                if g < 2:
                    nc.gpsimd.memset(avT[C : C + 1, :, :], 1.0)
                nc.vector.tensor_tensor(
                    out=avT[0:C, :, :],
                    in0=pav[:, :, 0:N],
                    in1=r2,
                    op=mybir.AluOpType.mult,
                )
                sg["avT"] = avT

            def emit_tail_b2(g):
                """projection + output staging + DMA for group g."""
                nonlocal ystage
                sg = st.pop(g)
                psy = ps_aux.tile([C, G * N], F32, name="psy", tag="aux")
                nc.tensor.matmul(out=psy, lhsT=pw2_sb, rhs=sg["avT"])
                if g % 2 == 0:
                    ystage = ystage_pool.tile([C, 2, G * N], BF16)
                nc.vector.tensor_copy(ystage[:, g % 2, :], psy)
                if g % 2 == 1:
                    blk = g // 2
                    nc.sync.dma_start(
                        out=out[:, blk * YB * N : (blk + 1) * YB * N],
                        in_=ystage,
                    )

            # Software pipeline: group g's tail work runs one full
            # iteration after its logits, so every tail dependency
            # (exp -> Pool mult -> denom, Ln -> Exp -> avT) has a whole
            # group of slack. Within an iteration, tails are emitted
            # FIRST so the ACT/DVE queues service the ops that gate the
            # tensor engine (Ln/Exp, avT, cast) before the new exps.
            emit_head_a(0)
            emit_head_b(0)
            emit_head_a(1)
            emit_head_b(1)
            emit_logits(0)
            for it in range(1, NG):
                emit_tail_a(it - 1)
                emit_head_a(it + 1) if it + 1 < NG else None
                emit_tail_b1(it - 1)
                emit_head_b(it + 1) if it + 1 < NG else None
                emit_tail_b2(it - 1)
                emit_logits(it)
            emit_tail_a(NG - 1)
            emit_tail_b1(NG - 1)
            emit_tail_b2(NG - 1)

    if split_waits:
        _split_sync_waits(nc)
    return nc


def _get_program():
    global _PROGRAM
    if _PROGRAM is None:
        _PROGRAM = _build_program()
    return _PROGRAM


# ------------------------------------------------------------------- kernel
def _core_instance_bidx(c):
    """B_ indices for core c's 128 window-instances, in device order."""
    w = np.arange(NI)
    return 512 * (w % 2) + NJ * c + (w // 2)


def _prepare_in_maps(x, y, mask, qkv_w, rpb_table, proj_w, proj_b):
    x = np.asarray(x, dtype=np.float32)
    y = np.asarray(y, dtype=np.float32)
    mask = np.asarray(mask, dtype=np.float32)
    qkv_w = np.asarray(qkv_w, dtype=np.float64)
    rpb_table = np.asarray(rpb_table, dtype=np.float32)
    proj_w = np.asarray(proj_w, dtype=np.float32)
    proj_b = np.asarray(proj_b, dtype=np.float32)

    scale = float(HD) ** -0.5

    # multiplicative mask+bias table: emb[tk, j, h*98+tq] = exp(mask+bias)
    bias = rpb_table[REL_IDX.reshape(-1)].reshape(N, N, H).transpose(2, 0, 1)
    emb_all = np.exp(mask[:, None, :, :] + bias[None, :, :, :])
    emb_t = np.ascontiguousarray(emb_all.transpose(3, 0, 1, 2)).reshape(
        N, NW, HB
    )

    # per-head composite: zw[:, h, :] = scale * Wq_h^T @ Wk_h  (96x96)
    zw_h = np.empty((C, H, C), dtype=np.float64)
    for h in range(H):
        wq_h = qkv_w[h * HD : (h + 1) * HD, :]            # (32, 96)
        wk_h = qkv_w[C + h * HD : C + (h + 1) * HD, :]    # (32, 96)
        zw_h[:, h, :] = scale * (wq_h.T @ wk_h)
    zw_h = zw_h.astype(NPBF16)

    wv_h = np.ascontiguousarray(qkv_w[2 * C : 3 * C].astype(np.float32).T
                                ).astype(NPBF16)
    pw2_h = np.empty((C + 1, C), dtype=np.float32)
    pw2_h[0:C] = proj_w.T
    pw2_h[C] = proj_b
    pw2_h = pw2_h.astype(NPBF16)

    in_maps = []
    bidx = []
    for c in range(NCORES):
        bi = _core_instance_bidx(c)
        bidx.append(bi)
        xc = x[bi].reshape(T, C)
        yc = y[bi].reshape(T, C)
        emb_c = np.ascontiguousarray(
            emb_t[:, NJ * c : NJ * (c + 1), :]
        ).astype(NPBF16)
        in_maps.append(
            {
                "xT": np.ascontiguousarray(xc.T).astype(NPBF16),
                "yT": np.ascontiguousarray(yc.T).astype(NPBF16),
                "emb": emb_c,
                "zw": zw_h,
                "wv": wv_h,
                "pw2": pw2_h,
            }
        )
    return in_maps, bidx


def kernel(x, y, mask, qkv_w, rpb_table, proj_w, proj_b):
    in_maps, bidx = _prepare_in_maps(
        x, y, mask, qkv_w, rpb_table, proj_w, proj_b
    )
    nc = _get_program()
    res = run_bass_kernel_spmd(nc, in_maps, list(range(NCORES)))

    out_full = np.empty((BWIN, N, C), dtype=np.float32)
    for c in range(NCORES):
        yt_o = np.asarray(res.results[c]["yT_out"]).astype(np.float32)
        out_full[bidx[c]] = yt_o.T.reshape(NI, N, C)
    return out_full


# revision 27
# speedup vs baseline: 1.1246x; 1.1122x over previous
"""Bass/Trainium2 kernel for nn_CrossWindowAttention3D (8-core SPMD).

Strategy (hardcoded for shapes B_=1024, N=98, C=96, H=3, NW=512):
- Shard 1024 window-instances over 8 cores: core c owns distinct windows
  [64c, 64c+64) for both batch replicas, interleaved (b0,j),(b1,j) so the
  exp(mask+bias) table for window j is loaded once per pair.
- Host precomputes channel-major bf16 transposes of x/y, folds the qk scale
  into the Q weights, and merges mask + relative-position bias into a single
  multiplicative table emb = exp(mask + bias) so the device softmax is
  exp(qk) * emb with no additive masking pass.
- Device computes, per window: qT/kT projections (batched over 4 windows),
  token-major v, transposed attention logits attnT = k_h q_h^T via three
  row-tiled matmuls (heads run concurrently in the PE array), exp on ACT,
  one multiply by emb, unnormalized head outputs + softmax denominators via
  col-tiled matmuls (an all-ones stationary broadcasts the denominators to
  all 96 channel partitions), reciprocal on DVE, one normalize multiply, and
  a channel-major projection with bias applied during the PSUM->SBUF copy.
- Output is returned channel-major [96, 12544] per core; host transposes.
"""

import sys

sys.path.insert(0, "/opt/trn_rl_repo")

import numpy as np
import ml_dtypes

import concourse.bass as bass
import concourse.tile as tile
from concourse import mybir
from concourse.vector_clock import ScopedClock
from concourse.bass_utils import run_bass_kernel_spmd

BF16 = mybir.dt.bfloat16
F32 = mybir.dt.float32
NPBF16 = ml_dtypes.bfloat16

WS = (2, 7, 7)
N = 98            # tokens per window
C = 96            # embed dim
H = 3             # heads
HD = 32           # head dim
NW = 512          # distinct windows
BWIN = 1024       # window-instances total
NCORES = 8
NI = 128          # instances per core
NJ = 64           # distinct windows per core
T = NI * N        # tokens per core = 12544
HB = H * N        # 294


# ---------------------------------------------------------------- tile patch
def _patch_tile_tail_drain():
    """This neuronxcc build rejects >1 sync wait on CTRL-class (Drain)
    instructions; split the TileContext tail-drain waits across NOPs."""
    if getattr(tile.TileContext, "_drain_patch_applied", False):
        return

    def _drain_and_barrier_split(self, tick_clock, wait_clock):
        nc = self.nc
        carrier = nc.sync.nop(nofuse=True)
        wait_clock.add_sem_waits(
            carrier.ins, ScopedClock({None: tick_clock.global_clock})
        )
        si = carrier.ins.sync_info
        waits = list(si.on_wait or []) if si is not None else []
        if len(waits) > 1:
            si.on_wait = waits[:1]
            for w in waits[1:]:
                extra = nc.sync.nop(nofuse=True)
                esi = extra.ins.sync_info
                if esi is None:
                    extra.ins.sync_info = mybir.SyncInfo(
                        on_wait=[w], on_update=[]
                    )
                else:
                    esi.on_wait = list(esi.on_wait or []) + [w]
        nc.sync.drain()
        nc.all_engine_barrier()
        assert self.sems is not None
        popped = nc._tile_sem_poison_stack.pop()
        assert popped is self._sem_poison
        nc.clear_and_free_semaphores(list(self.sems.allocated().values()))
        nc.all_engine_barrier()

    tile.TileContext._drain_and_barrier = _drain_and_barrier_split
    tile.TileContext._drain_patch_applied = True


def _split_sync_waits(nc, max_waits=1):
    """This neuronxcc build accepts at most one sync wait per instruction.
    Hoist excess waits onto same-engine NOPs inserted just before the
    instruction (the sequencer blocks on them in order; AND-semantics of
    multiple waits is preserved)."""
    ctr = 0
    for bb in nc.main_func.blocks:
        new_list = []
        changed = False
        for inst in bb.instructions:
            si = inst.sync_info
            waits = list(si.on_wait or []) if si is not None else []
            if len(waits) > max_waits:
                si.on_wait = waits[: max_waits]
                for w in waits[max_waits:]:
                    nop = mybir.InstNoOp(
                        name=f"I-waitsplit-{ctr}", ins=[], outs=[]
                    )
                    ctr += 1
                    nop.engine = inst.engine
                    nop.sync_info = mybir.SyncInfo(on_wait=[w], on_update=[])
                    new_list.append(nop)
                changed = True
            new_list.append(inst)
        if changed:
            bb.instructions = new_list


# ------------------------------------------------------------- host helpers
def _relative_position_index():
    ws = WS
    coords = np.stack(
        np.meshgrid(
            np.arange(ws[0]), np.arange(ws[1]), np.arange(ws[2]), indexing="ij"
        )
    )
    cf = coords.reshape(3, -1)
    rel = cf[:, :, None] - cf[:, None, :]
    rel = rel.transpose(1, 2, 0).astype(np.int64)
    rel[..., 0] += ws[0] - 1
    rel[..., 1] += ws[1] - 1
    rel[..., 2] += ws[2] - 1
    rel[..., 0] *= (2 * ws[1] - 1) * (2 * ws[2] - 1)
    rel[..., 1] *= 2 * ws[2] - 1
    return rel.sum(-1)  # (N, N)


REL_IDX = _relative_position_index()


# ------------------------------------------------------------ device program
_PROGRAM = None

# tiling knobs
XCH = 32          # instances per x/y SBUF chunk (4 chunks)
ECH = 8           # emb pairs per SBUF chunk (8 chunks)
G4 = 4            # instances per q/k projection batch & proj psum batch
YB = 8            # instances per output staging buffer / DMA


def _build_program(split_waits=True, n_pairs=NI // 2):
    _patch_tile_tail_drain()
    nc = bass.Bass()

    xT = nc.declare_dram_parameter("xT", [C, T], BF16, isOutput=False)
    yT = nc.declare_dram_parameter("yT", [C, T], BF16, isOutput=False)
    emb = nc.declare_dram_parameter("emb", [N, NJ, HB], BF16, isOutput=False)
    # per-head masked q weights: wqm[:, h, ci] = scale*qkv_w[ci, cj] if ci in
    # head h else 0.  Lets QK run as one full-K matmul per window (row-tiled
    # matmuls -- lhsT/rhs at partition offset -- crash this NRT build).
    wqm = nc.declare_dram_parameter("wqm", [C, H, C], BF16, isOutput=False)
    wk = nc.declare_dram_parameter("wk", [C, C], BF16, isOutput=False)
    wv = nc.declare_dram_parameter("wv", [C, C], BF16, isOutput=False)
    pw = nc.declare_dram_parameter("pw", [C, C], BF16, isOutput=False)
    pb = nc.declare_dram_parameter("pb", [C, 1], F32, isOutput=False)
    out = nc.declare_dram_parameter("yT_out", [C, T], F32, isOutput=True)

    from contextlib import ExitStack

    with tile.TileContext(nc) as tc:
        with ExitStack() as ctx:
            singles = ctx.enter_context(tc.tile_pool(name="singles", bufs=1))
            xt_pool = ctx.enter_context(tc.tile_pool(name="xt", bufs=2))
            yt_pool = ctx.enter_context(tc.tile_pool(name="yt", bufs=2))
            emb_pool = ctx.enter_context(tc.tile_pool(name="emb", bufs=2))
            qt_pool = ctx.enter_context(tc.tile_pool(name="qt", bufs=3))
            kt_pool = ctx.enter_context(tc.tile_pool(name="kt", bufs=3))
            v_pool = ctx.enter_context(tc.tile_pool(name="v", bufs=3))
            exp_pool = ctx.enter_context(tc.tile_pool(name="exp", bufs=3))
            expT_pool = ctx.enter_context(tc.tile_pool(name="expT", bufs=4))
            r2_pool = ctx.enter_context(tc.tile_pool(name="r2", bufs=3))
            attT_pool = ctx.enter_context(tc.tile_pool(name="attT", bufs=4))
            ystage_pool = ctx.enter_context(
                tc.tile_pool(name="ystage", bufs=2)
            )
            ps_qmk = ctx.enter_context(
                tc.tile_pool(name="ps_qmk", bufs=1, space="PSUM")
            )
            ps_v = ctx.enter_context(
                tc.tile_pool(name="ps_v", bufs=1, space="PSUM")
            )
            ps_qk = ctx.enter_context(
                tc.tile_pool(name="ps_qk", bufs=1, space="PSUM")
            )
            ps_av = ctx.enter_context(
                tc.tile_pool(name="ps_av", bufs=2, space="PSUM")
            )
            ps_y = ctx.enter_context(
                tc.tile_pool(name="ps_y", bufs=1, space="PSUM")
            )
            wqm_sb = singles.tile([C, H, C], BF16)
            nc.sync.dma_start(out=wqm_sb, in_=wqm[:, :, :])
            wk_sb = singles.tile([C, C], BF16)
            nc.sync.dma_start(out=wk_sb, in_=wk[:, :])
            wv_sb = singles.tile([C, C], BF16)
            nc.sync.dma_start(out=wv_sb, in_=wv[:, :])
            pw_sb = singles.tile([C, C], BF16)
            nc.sync.dma_start(out=pw_sb, in_=pw[:, :])
            pb_sb = singles.tile([C, 1], F32)
            nc.sync.dma_start(out=pb_sb, in_=pb[:, :])
            ones_sb = singles.tile([N, HD], BF16)
            nc.vector.memset(ones_sb, 1.0)

            xt_ch = yt_ch = emb_ch = None
            qt_g = kt_g = psy = ystage = None

            for pair in range(n_pairs):
                w0 = 2 * pair
                if w0 % XCH == 0:
                    ch = w0 // XCH
                    xt_ch = xt_pool.tile([C, XCH * N], BF16)
                    nc.sync.dma_start(
                        out=xt_ch, in_=xT[:, ch * XCH * N : (ch + 1) * XCH * N]
                    )
                    yt_ch = yt_pool.tile([C, XCH * N], BF16)
                    nc.sync.dma_start(
                        out=yt_ch, in_=yT[:, ch * XCH * N : (ch + 1) * XCH * N]
                    )
                if pair % ECH == 0:
                    ek = pair // ECH
                    emb_ch = emb_pool.tile([N, ECH, HB], BF16)
                    nc.sync.dma_start(
                        out=emb_ch, in_=emb[:, ek * ECH : (ek + 1) * ECH, :]
                    )

                if w0 % G4 == 0:
                    # q (per-head masked) / k projections for w0 .. w0+3
                    goff = (w0 % XCH) * N
                    # [C, H, 512]: head blocks padded to one PSUM bank each
                    pq = ps_qmk.tile([C, H, 512], F32, name="pq", tag="qmk")
                    for h in range(H):
                        nc.tensor.matmul(
                            out=pq[:, h, 0 : G4 * N],
                            lhsT=wqm_sb[:, h, :],
                            rhs=yt_ch[:, goff : goff + G4 * N],
                        )
                    qt_g = qt_pool.tile([C, H, G4 * N], BF16)
                    nc.vector.tensor_copy(qt_g, pq[:, :, 0 : G4 * N])
                    pk = ps_qmk.tile([C, 512], F32, name="pk", tag="qmk")
                    nc.tensor.matmul(
                        out=pk[:, 0 : G4 * N],
                        lhsT=wk_sb,
                        rhs=xt_ch[:, goff : goff + G4 * N],
                    )
                    kt_g = kt_pool.tile([C, G4 * N], BF16)
                    nc.vector.tensor_copy(kt_g, pk[:, 0 : G4 * N])

                # ---- v projections, batched 4 windows per psum tile/copy
                if w0 % G4 == 0:
                    pv4 = ps_v.tile([N, G4, 128], F32)
                    for j in range(G4):
                        col = ((w0 + j) % XCH) * N
                        nc.tensor.matmul(
                            out=pv4[:, j, 0:C],
                            lhsT=xt_ch[:, col : col + N],
                            rhs=wv_sb,
                        )
                    v4_sb = v_pool.tile([N, G4, C], BF16)
                    nc.scalar.copy(v4_sb, pv4[:, :, 0:C])

                # ---- qk logits + exp per window
                exp_pair = exp_pool.tile([N, 2, HB], BF16)
                for k in range(2):
                    w = w0 + k
                    i4 = (w % G4) * N
                    pqk = ps_qk.tile([N, 512], F32)
                    nc.tensor.matmul(
                        out=pqk[:, 0:HB],
                        lhsT=kt_g[:, i4 : i4 + N],
                        rhs=qt_g[:, :, i4 : i4 + N],
                    )
                    nc.scalar.activation(
                        out=exp_pair[:, k, :],
                        in_=pqk[:, 0:HB],
                        func=mybir.ActivationFunctionType.Exp,
                    )

                # ---- one multiply by emb for the pair (same distinct window)
                pj = pair % ECH
                expT = expT_pool.tile([N, 2, HB], BF16)
                emb_b = emb_ch[:, pj : pj + 1, :].broadcast_to((N, 2, HB))
                nc.vector.tensor_tensor(
                    out=expT, in0=exp_pair, in1=emb_b, op=mybir.AluOpType.mult
                )

                # ---- denominators: one [C, 4N] psum tile per 4 windows
                if pair % 2 == 0:
                    pdbc = ps_av.tile([C, 512], F32, name="pdbc", tag="avdbc")
                doff = (pair % 2) * 2 * N
                for h in range(H):
                    nc.tensor.matmul(
                        out=pdbc[h * HD : (h + 1) * HD, doff : doff + 2 * N],
                        lhsT=ones_sb,
                        rhs=expT[:, :, h * N : (h + 1) * N],
                    )
                if pair % 2 == 1:
                    # 1/d = exp(-ln(d)); Ln+Exp share one ACT table set
                    t_ln = r2_pool.tile([C, 4 * N], F32, name="t_ln", tag="tl")
                    nc.scalar.activation(
                        out=t_ln,
                        in_=pdbc[:, 0 : 4 * N],
                        func=mybir.ActivationFunctionType.Ln,
                    )
                    r2 = r2_pool.tile([C, 4 * N], F32, name="r2", tag="r2")
                    nc.scalar.activation(
                        out=r2,
                        in_=t_ln,
                        func=mybir.ActivationFunctionType.Exp,
                        scale=-1.0,
                    )
                    # ---- av + norm + proj for the 4 windows of this group
                    g0 = w0 - 2
                    psy = ps_y.tile([C, 512], F32)
                    for kk in range(2):
                        ep = expT_prev if kk == 0 else expT
                        pav = ps_av.tile([C, 512], F32, name="pav", tag="avdbc")
                        for k in range(2):
                            j = 2 * kk + k
                            for h in range(H):
                                nc.tensor.matmul(
                                    out=pav[
                                        h * HD : (h + 1) * HD,
                                        k * N : (k + 1) * N,
                                    ],
                                    lhsT=v4_sb[:, j, h * HD : (h + 1) * HD],
                                    rhs=ep[:, k, h * N : (h + 1) * N],
                                )
                        attT = attT_pool.tile([C, 2 * N], BF16)
                        nc.vector.tensor_tensor(
                            out=attT,
                            in0=pav[:, 0 : 2 * N],
                            in1=r2[:, kk * 2 * N : (kk + 1) * 2 * N],
                            op=mybir.AluOpType.mult,
                        )
                        for k in range(2):
                            j = 2 * kk + k
                            nc.tensor.matmul(
                                out=psy[:, j * N : (j + 1) * N],
                                lhsT=pw_sb,
                                rhs=attT[:, k * N : (k + 1) * N],
                            )
                    # bias add during PSUM->SBUF staging, then DMA out per 8
                    if (g0 // G4) % 2 == 0:
                        ystage = ystage_pool.tile([C, YB * N], F32)
                    yoff = ((g0 // G4) % 2) * G4 * N
                    nc.scalar.activation(
                        out=ystage[:, yoff : yoff + G4 * N],
                        in_=psy[:, 0 : G4 * N],
                        func=mybir.ActivationFunctionType.Identity,
                        bias=pb_sb,
                    )
                    if (g0 + G4) % YB == 0:
                        blk = g0 // YB
                        nc.sync.dma_start(
                            out=out[:, blk * YB * N : (blk + 1) * YB * N],
                            in_=ystage,
                        )
                expT_prev = expT
    if split_waits:
        _split_sync_waits(nc)
    return nc


def _get_program():
    global _PROGRAM
    if _PROGRAM is None:
        _PROGRAM = _build_program()
    return _PROGRAM


# ------------------------------------------------------------------- kernel
def _core_instance_bidx(c):
    """B_ indices for core c's 128 window-instances, in device order."""
    w = np.arange(NI)
    return 512 * (w % 2) + NJ * c + (w // 2)


def _prepare_in_maps(x, y, mask, qkv_w, rpb_table, proj_w, proj_b):
    x = np.asarray(x, dtype=np.float32)
    y = np.asarray(y, dtype=np.float32)
    mask = np.asarray(mask, dtype=np.float32)
    qkv_w = np.asarray(qkv_w, dtype=np.float32)
    rpb_table = np.asarray(rpb_table, dtype=np.float32)
    proj_w = np.asarray(proj_w, dtype=np.float32)
    proj_b = np.asarray(proj_b, dtype=np.float32)

    scale = float(HD) ** -0.5

    # emb[wg, h, tq, tk] = exp(mask[wg, tq, tk] + bias[h, tq, tk])
    bias = rpb_table[REL_IDX.reshape(-1)].reshape(N, N, H).transpose(2, 0, 1)
    emb_all = np.exp(mask[:, None, :, :] + bias[None, :, :, :])
    # device layout [tk, wg, h*98+tq]
    emb_t = np.ascontiguousarray(emb_all.transpose(3, 0, 1, 2)).reshape(
        N, NW, HB
    )

    wq_t = (scale * qkv_w[0:C]).T  # [cj, ci]
    wqm_h = np.zeros((C, H, C), dtype=np.float32)
    for h in range(H):
        wqm_h[:, h, h * HD : (h + 1) * HD] = wq_t[:, h * HD : (h + 1) * HD]
    wqm_h = wqm_h.astype(NPBF16)
    wk_h = np.ascontiguousarray(qkv_w[C : 2 * C].T).astype(NPBF16)
    wv_h = np.ascontiguousarray(qkv_w[2 * C : 3 * C].T).astype(NPBF16)
    pw_h = np.ascontiguousarray(proj_w.T).astype(NPBF16)
    pb_h = np.ascontiguousarray(proj_b.reshape(C, 1)).astype(np.float32)

    in_maps = []
    bidx = []
    for c in range(NCORES):
        bi = _core_instance_bidx(c)
        bidx.append(bi)
        xc = x[bi].reshape(T, C)
        yc = y[bi].reshape(T, C)
        emb_c = np.ascontiguousarray(
            emb_t[:, NJ * c : NJ * (c + 1), :]
        ).astype(NPBF16)
        in_maps.append(
            {
                "xT": np.ascontiguousarray(xc.T).astype(NPBF16),
                "yT": np.ascontiguousarray(yc.T).astype(NPBF16),
                "emb": emb_c,
                "wqm": wqm_h,
                "wk": wk_h,
                "wv": wv_h,
                "pw": pw_h,
                "pb": pb_h,
            }
        )
    return in_maps, bidx


def kernel(x, y, mask, qkv_w, rpb_table, proj_w, proj_b):
    in_maps, bidx = _prepare_in_maps(
        x, y, mask, qkv_w, rpb_table, proj_w, proj_b
    )
    nc = _get_program()
    res = run_bass_kernel_spmd(nc, in_maps, list(range(NCORES)))

    out_full = np.empty((BWIN, N, C), dtype=np.float32)
    for c in range(NCORES):
        yt_o = np.asarray(res.results[c]["yT_out"], dtype=np.float32)
        out_full[bidx[c]] = yt_o.T.reshape(NI, N, C)
    return out_full



# revision 37
# speedup vs baseline: 1.3076x; 1.1627x over previous
"""Bass/Trainium2 kernel for nn_CrossWindowAttention3D (8-core SPMD).

Strategy (hardcoded for shapes B_=1024, N=98, C=96, H=3, NW=512):
- Shard 1024 window-instances over 8 cores: core c owns distinct windows
  [64c, 64c+64) for both batch replicas, interleaved (b0,j),(b1,j) so the
  exp(mask+bias) table for window j is loaded once per pair.
- Host precomputes channel-major bf16 transposes of x/y, folds the qk scale
  into the Q weights, and merges mask + relative-position bias into a single
  multiplicative table emb = exp(mask + bias) so the device softmax is
  exp(qk) * emb with no additive masking pass.
- Device computes, per window: qT/kT projections (batched over 4 windows),
  token-major v, transposed attention logits attnT = k_h q_h^T via three
  row-tiled matmuls (heads run concurrently in the PE array), exp on ACT,
  one multiply by emb, unnormalized head outputs + softmax denominators via
  col-tiled matmuls (an all-ones stationary broadcasts the denominators to
  all 96 channel partitions), reciprocal on DVE, one normalize multiply, and
  a channel-major projection with bias applied during the PSUM->SBUF copy.
- Output is returned channel-major [96, 12544] per core; host transposes.
"""

import sys

sys.path.insert(0, "/opt/trn_rl_repo")

import numpy as np
import ml_dtypes

import concourse.bass as bass
import concourse.tile as tile
from concourse import mybir
from concourse.vector_clock import ScopedClock
from concourse.bass_utils import run_bass_kernel_spmd

BF16 = mybir.dt.bfloat16
F32 = mybir.dt.float32
NPBF16 = ml_dtypes.bfloat16

WS = (2, 7, 7)
N = 98            # tokens per window
C = 96            # embed dim
H = 3             # heads
HD = 32           # head dim
NW = 512          # distinct windows
BWIN = 1024       # window-instances total
NCORES = 8
NI = 128          # instances per core
NJ = 64           # distinct windows per core
T = NI * N        # tokens per core = 12544
HB = H * N        # 294


# ---------------------------------------------------------------- tile patch
def _patch_tile_tail_drain():
    """This neuronxcc build rejects >1 sync wait on CTRL-class (Drain)
    instructions; split the TileContext tail-drain waits across NOPs."""
    if getattr(tile.TileContext, "_drain_patch_applied", False):
        return

    def _drain_and_barrier_split(self, tick_clock, wait_clock):
        nc = self.nc
        carrier = nc.sync.nop(nofuse=True)
        wait_clock.add_sem_waits(
            carrier.ins, ScopedClock({None: tick_clock.global_clock})
        )
        si = carrier.ins.sync_info
        waits = list(si.on_wait or []) if si is not None else []
        if len(waits) > 1:
            si.on_wait = waits[:1]
            for w in waits[1:]:
                extra = nc.sync.nop(nofuse=True)
                esi = extra.ins.sync_info
                if esi is None:
                    extra.ins.sync_info = mybir.SyncInfo(
                        on_wait=[w], on_update=[]
                    )
                else:
                    esi.on_wait = list(esi.on_wait or []) + [w]
        nc.sync.drain()
        nc.all_engine_barrier()
        assert self.sems is not None
        popped = nc._tile_sem_poison_stack.pop()
        assert popped is self._sem_poison
        nc.clear_and_free_semaphores(list(self.sems.allocated().values()))
        nc.all_engine_barrier()

    tile.TileContext._drain_and_barrier = _drain_and_barrier_split
    tile.TileContext._drain_patch_applied = True


def _split_sync_waits(nc, max_waits=1):
    """This neuronxcc build accepts at most one sync wait per instruction.
    Hoist excess waits onto same-engine NOPs inserted just before the
    instruction (the sequencer blocks on them in order; AND-semantics of
    multiple waits is preserved)."""
    ctr = 0
    for bb in nc.main_func.blocks:
        new_list = []
        changed = False
        for inst in bb.instructions:
            si = inst.sync_info
            waits = list(si.on_wait or []) if si is not None else []
            if len(waits) > max_waits:
                si.on_wait = waits[: max_waits]
                for w in waits[max_waits:]:
                    nop = mybir.InstNoOp(
                        name=f"I-waitsplit-{ctr}", ins=[], outs=[]
                    )
                    ctr += 1
                    nop.engine = inst.engine
                    nop.sync_info = mybir.SyncInfo(on_wait=[w], on_update=[])
                    new_list.append(nop)
                changed = True
            new_list.append(inst)
        if changed:
            bb.instructions = new_list


# ------------------------------------------------------------- host helpers
def _relative_position_index():
    ws = WS
    coords = np.stack(
        np.meshgrid(
            np.arange(ws[0]), np.arange(ws[1]), np.arange(ws[2]), indexing="ij"
        )
    )
    cf = coords.reshape(3, -1)
    rel = cf[:, :, None] - cf[:, None, :]
    rel = rel.transpose(1, 2, 0).astype(np.int64)
    rel[..., 0] += ws[0] - 1
    rel[..., 1] += ws[1] - 1
    rel[..., 2] += ws[2] - 1
    rel[..., 0] *= (2 * ws[1] - 1) * (2 * ws[2] - 1)
    rel[..., 1] *= 2 * ws[2] - 1
    return rel.sum(-1)  # (N, N)


REL_IDX = _relative_position_index()


# ------------------------------------------------------------ device program
_PROGRAM = None

# tiling knobs
XCH = 32          # instances per x/y SBUF chunk (4 chunks)
ECH = 8           # emb pairs per SBUF chunk (8 chunks)
G4 = 4            # instances per q/k projection batch & proj psum batch
YB = 8            # instances per output staging buffer / DMA


def _build_program(split_waits=True, n_pairs=NI // 2):
    _patch_tile_tail_drain()
    nc = bass.Bass()

    xT = nc.declare_dram_parameter("xT", [C, T], BF16, isOutput=False)
    yT = nc.declare_dram_parameter("yT", [C, T], BF16, isOutput=False)
    emb = nc.declare_dram_parameter("emb", [N, NJ, HB], BF16, isOutput=False)
    # plain scaled q weights; qk runs as 3 per-head matmuls with lhsT/rhs
    # at partition offset h*HD (verified working on this NRT build).
    wq = nc.declare_dram_parameter("wq", [C, C], BF16, isOutput=False)
    wk = nc.declare_dram_parameter("wk", [C, C], BF16, isOutput=False)
    wv = nc.declare_dram_parameter("wv", [C, C], BF16, isOutput=False)
    pw = nc.declare_dram_parameter("pw", [C, C], BF16, isOutput=False)
    pb = nc.declare_dram_parameter("pb", [C, 1], F32, isOutput=False)
    out = nc.declare_dram_parameter("yT_out", [C, T], F32, isOutput=True)

    from contextlib import ExitStack

    with tile.TileContext(nc) as tc:
        with ExitStack() as ctx:
            singles = ctx.enter_context(tc.tile_pool(name="singles", bufs=1))
            xt_pool = ctx.enter_context(tc.tile_pool(name="xt", bufs=2))
            yt_pool = ctx.enter_context(tc.tile_pool(name="yt", bufs=2))
            emb_pool = ctx.enter_context(tc.tile_pool(name="emb", bufs=2))
            qt_pool = ctx.enter_context(tc.tile_pool(name="qt", bufs=3))
            kt_pool = ctx.enter_context(tc.tile_pool(name="kt", bufs=3))
            v_pool = ctx.enter_context(tc.tile_pool(name="v", bufs=3))
            exp_pool = ctx.enter_context(tc.tile_pool(name="exp", bufs=3))
            expT_pool = ctx.enter_context(tc.tile_pool(name="expT", bufs=4))
            r2_pool = ctx.enter_context(tc.tile_pool(name="r2", bufs=3))
            attT_pool = ctx.enter_context(tc.tile_pool(name="attT", bufs=4))
            ystage_pool = ctx.enter_context(
                tc.tile_pool(name="ystage", bufs=2)
            )
            ps_qmk = ctx.enter_context(
                tc.tile_pool(name="ps_qmk", bufs=1, space="PSUM")
            )
            ps_v = ctx.enter_context(
                tc.tile_pool(name="ps_v", bufs=1, space="PSUM")
            )
            ps_qk = ctx.enter_context(
                tc.tile_pool(name="ps_qk", bufs=2, space="PSUM")
            )
            ps_av = ctx.enter_context(
                tc.tile_pool(name="ps_av", bufs=2, space="PSUM")
            )
            ps_y = ctx.enter_context(
                tc.tile_pool(name="ps_y", bufs=1, space="PSUM")
            )
            wq_sb = singles.tile([C, C], BF16)
            nc.sync.dma_start(out=wq_sb, in_=wq[:, :])
            wk_sb = singles.tile([C, C], BF16)
            nc.sync.dma_start(out=wk_sb, in_=wk[:, :])
            wv_sb = singles.tile([C, C], BF16)
            nc.sync.dma_start(out=wv_sb, in_=wv[:, :])
            pw_sb = singles.tile([C, C], BF16)
            nc.sync.dma_start(out=pw_sb, in_=pw[:, :])
            pb_sb = singles.tile([C, 1], F32)
            nc.sync.dma_start(out=pb_sb, in_=pb[:, :])
            ones_sb = singles.tile([N, HD], BF16)
            nc.vector.memset(ones_sb, 1.0)
            # persistent masked-q staging buffers: only the per-head
            # diagonal blocks are ever (re)written by the casts below,
            # off-head blocks stay zero from this one-time memset.
            qtbufs = []
            for i in range(3):
                qb = singles.tile([C, H, G4 * N], BF16, name=f"qtb{i}")
                nc.gpsimd.memset(qb, 0.0)
                qtbufs.append(qb)

            xt_ch = yt_ch = emb_ch = None
            qt_g = kt_g = psy = ystage = None

            for pair in range(n_pairs):
                w0 = 2 * pair
                if w0 % XCH == 0:
                    ch = w0 // XCH
                    xt_ch = xt_pool.tile([C, XCH * N], BF16)
                    nc.sync.dma_start(
                        out=xt_ch, in_=xT[:, ch * XCH * N : (ch + 1) * XCH * N]
                    )
                    yt_ch = yt_pool.tile([C, XCH * N], BF16)
                    nc.sync.dma_start(
                        out=yt_ch, in_=yT[:, ch * XCH * N : (ch + 1) * XCH * N]
                    )
                if pair % ECH == 0:
                    ek = pair // ECH
                    emb_ch = emb_pool.tile([N, ECH, HB], BF16)
                    nc.sync.dma_start(
                        out=emb_ch, in_=emb[:, ek * ECH : (ek + 1) * ECH, :]
                    )

                if w0 % G4 == 0:
                    # q / k projections for w0 .. w0+3 (one matmul each)
                    goff = (w0 % XCH) * N
                    pq = ps_qmk.tile([C, 512], F32, name="pq", tag="pq")
                    nc.tensor.matmul(
                        out=pq[:, 0 : G4 * N],
                        lhsT=wq_sb,
                        rhs=yt_ch[:, goff : goff + G4 * N],
                    )
                    qt_g = qtbufs[(w0 // G4) % 3]
                    for h in range(H):
                        nc.vector.tensor_copy(
                            qt_g[h * HD : (h + 1) * HD, h, :],
                            pq[h * HD : (h + 1) * HD, 0 : G4 * N],
                        )
                    pk = ps_qmk.tile([C, 512], F32, name="pk", tag="pk")
                    nc.tensor.matmul(
                        out=pk[:, 0 : G4 * N],
                        lhsT=wk_sb,
                        rhs=xt_ch[:, goff : goff + G4 * N],
                    )
                    kt_g = kt_pool.tile([C, G4 * N], BF16)
                    nc.vector.tensor_copy(kt_g, pk[:, 0 : G4 * N])

                # ---- v projections, batched 4 windows per psum tile/copy
                if w0 % G4 == 0:
                    pv4 = ps_v.tile([N, G4, 128], F32)
                    for j in range(G4):
                        col = ((w0 + j) % XCH) * N
                        nc.tensor.matmul(
                            out=pv4[:, j, 0:C],
                            lhsT=xt_ch[:, col : col + N],
                            rhs=wv_sb,
                        )
                    v4_sb = v_pool.tile([N, G4, C], BF16)
                    nc.scalar.copy(v4_sb, pv4[:, :, 0:C])

                # ---- qk logits + exp per window
                exp_pair = exp_pool.tile([N, 2, HB], BF16)
                for k in range(2):
                    w = w0 + k
                    i4 = (w % G4) * N
                    pqk = ps_qk.tile([N, 512], F32)
                    nc.tensor.matmul(
                        out=pqk[:, 0:HB],
                        lhsT=kt_g[:, i4 : i4 + N],
                        rhs=qt_g[:, :, i4 : i4 + N],
                    )
                    nc.scalar.activation(
                        out=exp_pair[:, k, :],
                        in_=pqk[:, 0:HB],
                        func=mybir.ActivationFunctionType.Exp,
                    )

                # ---- one multiply by emb for the pair (same distinct window)
                pj = pair % ECH
                expT = expT_pool.tile([N, 2, HB], BF16)
                emb_b = emb_ch[:, pj : pj + 1, :].broadcast_to((N, 2, HB))
                nc.vector.tensor_tensor(
                    out=expT, in0=exp_pair, in1=emb_b, op=mybir.AluOpType.mult
                )

                # ---- denominators: one [C, 4N] psum tile per 4 windows
                if pair % 2 == 0:
                    pdbc = ps_av.tile([C, 512], F32, name="pdbc", tag="avdbc")
                doff = (pair % 2) * 2 * N
                for h in range(H):
                    nc.tensor.matmul(
                        out=pdbc[h * HD : (h + 1) * HD, doff : doff + 2 * N],
                        lhsT=ones_sb,
                        rhs=expT[:, :, h * N : (h + 1) * N],
                    )
                if pair % 2 == 1:
                    # 1/d = exp(-ln(d)); Ln+Exp share one ACT table set
                    t_ln = r2_pool.tile([C, 4 * N], F32, name="t_ln", tag="tl")
                    nc.scalar.activation(
                        out=t_ln,
                        in_=pdbc[:, 0 : 4 * N],
                        func=mybir.ActivationFunctionType.Ln,
                    )
                    r2 = r2_pool.tile([C, 4 * N], F32, name="r2", tag="r2")
                    nc.scalar.activation(
                        out=r2,
                        in_=t_ln,
                        func=mybir.ActivationFunctionType.Exp,
                        scale=-1.0,
                    )
                    # ---- av + norm + proj for the 4 windows of this group
                    g0 = w0 - 2
                    psy = ps_y.tile([C, 512], F32)
                    for kk in range(2):
                        ep = expT_prev if kk == 0 else expT
                        pav = ps_av.tile([C, 512], F32, name="pav", tag="avdbc")
                        for k in range(2):
                            j = 2 * kk + k
                            for h in range(H):
                                nc.tensor.matmul(
                                    out=pav[
                                        h * HD : (h + 1) * HD,
                                        k * N : (k + 1) * N,
                                    ],
                                    lhsT=v4_sb[:, j, h * HD : (h + 1) * HD],
                                    rhs=ep[:, k, h * N : (h + 1) * N],
                                )
                        attT = attT_pool.tile([C, 2 * N], BF16)
                        nc.vector.tensor_tensor(
                            out=attT,
                            in0=pav[:, 0 : 2 * N],
                            in1=r2[:, kk * 2 * N : (kk + 1) * 2 * N],
                            op=mybir.AluOpType.mult,
                        )
                        for k in range(2):
                            j = 2 * kk + k
                            nc.tensor.matmul(
                                out=psy[:, j * N : (j + 1) * N],
                                lhsT=pw_sb,
                                rhs=attT[:, k * N : (k + 1) * N],
                            )
                    # bias add during PSUM->SBUF staging, then DMA out per 8
                    if (g0 // G4) % 2 == 0:
                        ystage = ystage_pool.tile([C, YB * N], F32)
                    yoff = ((g0 // G4) % 2) * G4 * N
                    nc.scalar.activation(
                        out=ystage[:, yoff : yoff + G4 * N],
                        in_=psy[:, 0 : G4 * N],
                        func=mybir.ActivationFunctionType.Identity,
                        bias=pb_sb,
                    )
                    if (g0 + G4) % YB == 0:
                        blk = g0 // YB
                        nc.sync.dma_start(
                            out=out[:, blk * YB * N : (blk + 1) * YB * N],
                            in_=ystage,
                        )
                expT_prev = expT
    if split_waits:
        _split_sync_waits(nc)
    return nc


def _get_program():
    global _PROGRAM
    if _PROGRAM is None:
        _PROGRAM = _build_program()
    return _PROGRAM


# ------------------------------------------------------------------- kernel
def _core_instance_bidx(c):
    """B_ indices for core c's 128 window-instances, in device order."""
    w = np.arange(NI)
    return 512 * (w % 2) + NJ * c + (w // 2)


def _prepare_in_maps(x, y, mask, qkv_w, rpb_table, proj_w, proj_b):
    x = np.asarray(x, dtype=np.float32)
    y = np.asarray(y, dtype=np.float32)
    mask = np.asarray(mask, dtype=np.float32)
    qkv_w = np.asarray(qkv_w, dtype=np.float32)
    rpb_table = np.asarray(rpb_table, dtype=np.float32)
    proj_w = np.asarray(proj_w, dtype=np.float32)
    proj_b = np.asarray(proj_b, dtype=np.float32)

    scale = float(HD) ** -0.5

    # emb[wg, h, tq, tk] = exp(mask[wg, tq, tk] + bias[h, tq, tk])
    bias = rpb_table[REL_IDX.reshape(-1)].reshape(N, N, H).transpose(2, 0, 1)
    emb_all = np.exp(mask[:, None, :, :] + bias[None, :, :, :])
    # device layout [tk, wg, h*98+tq]
    emb_t = np.ascontiguousarray(emb_all.transpose(3, 0, 1, 2)).reshape(
        N, NW, HB
    )

    wq_h = np.ascontiguousarray((scale * qkv_w[0:C]).T).astype(NPBF16)
    wk_h = np.ascontiguousarray(qkv_w[C : 2 * C].T).astype(NPBF16)
    wv_h = np.ascontiguousarray(qkv_w[2 * C : 3 * C].T).astype(NPBF16)
    pw_h = np.ascontiguousarray(proj_w.T).astype(NPBF16)
    pb_h = np.ascontiguousarray(proj_b.reshape(C, 1)).astype(np.float32)

    in_maps = []
    bidx = []
    for c in range(NCORES):
        bi = _core_instance_bidx(c)
        bidx.append(bi)
        xc = x[bi].reshape(T, C)
        yc = y[bi].reshape(T, C)
        emb_c = np.ascontiguousarray(
            emb_t[:, NJ * c : NJ * (c + 1), :]
        ).astype(NPBF16)
        in_maps.append(
            {
                "xT": np.ascontiguousarray(xc.T).astype(NPBF16),
                "yT": np.ascontiguousarray(yc.T).astype(NPBF16),
                "emb": emb_c,
                "wq": wq_h,
                "wk": wk_h,
                "wv": wv_h,
                "pw": pw_h,
                "pb": pb_h,
            }
        )
    return in_maps, bidx


def kernel(x, y, mask, qkv_w, rpb_table, proj_w, proj_b):
    in_maps, bidx = _prepare_in_maps(
        x, y, mask, qkv_w, rpb_table, proj_w, proj_b
    )
    nc = _get_program()
    res = run_bass_kernel_spmd(nc, in_maps, list(range(NCORES)))

    out_full = np.empty((BWIN, N, C), dtype=np.float32)
    for c in range(NCORES):
        yt_o = np.asarray(res.results[c]["yT_out"], dtype=np.float32)
        out_full[bidx[c]] = yt_o.T.reshape(NI, N, C)
    return out_full



# revision 38
# speedup vs baseline: 1.3110x; 1.0026x over previous
"""Bass/Trainium2 kernel for nn_CrossWindowAttention3D (8-core SPMD).

Strategy (hardcoded for shapes B_=1024, N=98, C=96, H=3, NW=512):
- Shard 1024 window-instances over 8 cores: core c owns distinct windows
  [64c, 64c+64) for both batch replicas, interleaved (b0,j),(b1,j) so the
  exp(mask+bias) table for window j is loaded once per pair.
- Host precomputes channel-major bf16 transposes of x/y, folds the qk scale
  into the Q weights, and merges mask + relative-position bias into a single
  multiplicative table emb = exp(mask + bias) so the device softmax is
  exp(qk) * emb with no additive masking pass.
- Device computes, per window: qT/kT projections (batched over 4 windows),
  token-major v, transposed attention logits attnT = k_h q_h^T via three
  row-tiled matmuls (heads run concurrently in the PE array), exp on ACT,
  one multiply by emb, unnormalized head outputs + softmax denominators via
  col-tiled matmuls (an all-ones stationary broadcasts the denominators to
  all 96 channel partitions), reciprocal on DVE, one normalize multiply, and
  a channel-major projection with bias applied during the PSUM->SBUF copy.
- Output is returned channel-major [96, 12544] per core; host transposes.
"""

import sys

sys.path.insert(0, "/opt/trn_rl_repo")

import numpy as np
import ml_dtypes

import concourse.bass as bass
import concourse.tile as tile
from concourse import mybir
from concourse.vector_clock import ScopedClock
from concourse.bass_utils import run_bass_kernel_spmd

BF16 = mybir.dt.bfloat16
F32 = mybir.dt.float32
NPBF16 = ml_dtypes.bfloat16

WS = (2, 7, 7)
N = 98            # tokens per window
C = 96            # embed dim
H = 3             # heads
HD = 32           # head dim
NW = 512          # distinct windows
BWIN = 1024       # window-instances total
NCORES = 8
NI = 128          # instances per core
NJ = 64           # distinct windows per core
T = NI * N        # tokens per core = 12544
HB = H * N        # 294


# ---------------------------------------------------------------- tile patch
def _patch_tile_tail_drain():
    """This neuronxcc build rejects >1 sync wait on CTRL-class (Drain)
    instructions; split the TileContext tail-drain waits across NOPs."""
    if getattr(tile.TileContext, "_drain_patch_applied", False):
        return

    def _drain_and_barrier_split(self, tick_clock, wait_clock):
        nc = self.nc
        carrier = nc.sync.nop(nofuse=True)
        wait_clock.add_sem_waits(
            carrier.ins, ScopedClock({None: tick_clock.global_clock})
        )
        si = carrier.ins.sync_info
        waits = list(si.on_wait or []) if si is not None else []
        if len(waits) > 1:
            si.on_wait = waits[:1]
            for w in waits[1:]:
                extra = nc.sync.nop(nofuse=True)
                esi = extra.ins.sync_info
                if esi is None:
                    extra.ins.sync_info = mybir.SyncInfo(
                        on_wait=[w], on_update=[]
                    )
                else:
                    esi.on_wait = list(esi.on_wait or []) + [w]
        nc.sync.drain()
        nc.all_engine_barrier()
        assert self.sems is not None
        popped = nc._tile_sem_poison_stack.pop()
        assert popped is self._sem_poison
        nc.clear_and_free_semaphores(list(self.sems.allocated().values()))
        nc.all_engine_barrier()

    tile.TileContext._drain_and_barrier = _drain_and_barrier_split
    tile.TileContext._drain_patch_applied = True


def _split_sync_waits(nc, max_waits=1):
    """This neuronxcc build accepts at most one sync wait per instruction.
    Hoist excess waits onto same-engine NOPs inserted just before the
    instruction (the sequencer blocks on them in order; AND-semantics of
    multiple waits is preserved)."""
    ctr = 0
    for bb in nc.main_func.blocks:
        new_list = []
        changed = False
        for inst in bb.instructions:
            si = inst.sync_info
            waits = list(si.on_wait or []) if si is not None else []
            if len(waits) > max_waits:
                si.on_wait = waits[: max_waits]
                for w in waits[max_waits:]:
                    nop = mybir.InstNoOp(
                        name=f"I-waitsplit-{ctr}", ins=[], outs=[]
                    )
                    ctr += 1
                    nop.engine = inst.engine
                    nop.sync_info = mybir.SyncInfo(on_wait=[w], on_update=[])
                    new_list.append(nop)
                changed = True
            new_list.append(inst)
        if changed:
            bb.instructions = new_list


# ------------------------------------------------------------- host helpers
def _relative_position_index():
    ws = WS
    coords = np.stack(
        np.meshgrid(
            np.arange(ws[0]), np.arange(ws[1]), np.arange(ws[2]), indexing="ij"
        )
    )
    cf = coords.reshape(3, -1)
    rel = cf[:, :, None] - cf[:, None, :]
    rel = rel.transpose(1, 2, 0).astype(np.int64)
    rel[..., 0] += ws[0] - 1
    rel[..., 1] += ws[1] - 1
    rel[..., 2] += ws[2] - 1
    rel[..., 0] *= (2 * ws[1] - 1) * (2 * ws[2] - 1)
    rel[..., 1] *= 2 * ws[2] - 1
    return rel.sum(-1)  # (N, N)


REL_IDX = _relative_position_index()


# ------------------------------------------------------------ device program
_PROGRAM = None

# tiling knobs
XCH = 32          # instances per x/y SBUF chunk (4 chunks)
ECH = 8           # emb pairs per SBUF chunk (8 chunks)
G4 = 4            # instances per q/k projection batch & proj psum batch
YB = 8            # instances per output staging buffer / DMA


def _build_program(split_waits=True, n_pairs=NI // 2):
    _patch_tile_tail_drain()
    nc = bass.Bass()

    xT = nc.declare_dram_parameter("xT", [C, T], BF16, isOutput=False)
    yT = nc.declare_dram_parameter("yT", [C, T], BF16, isOutput=False)
    emb = nc.declare_dram_parameter("emb", [N, NJ, HB], BF16, isOutput=False)
    # plain scaled q weights; qk runs as 3 per-head matmuls with lhsT/rhs
    # at partition offset h*HD (verified working on this NRT build).
    wq = nc.declare_dram_parameter("wq", [C, C], BF16, isOutput=False)
    wk = nc.declare_dram_parameter("wk", [C, C], BF16, isOutput=False)
    wv = nc.declare_dram_parameter("wv", [C, C], BF16, isOutput=False)
    pw = nc.declare_dram_parameter("pw", [C, C], BF16, isOutput=False)
    pb = nc.declare_dram_parameter("pb", [C, 1], F32, isOutput=False)
    out = nc.declare_dram_parameter("yT_out", [C, T], F32, isOutput=True)

    from contextlib import ExitStack

    with tile.TileContext(nc) as tc:
        with ExitStack() as ctx:
            singles = ctx.enter_context(tc.tile_pool(name="singles", bufs=1))
            xt_pool = ctx.enter_context(tc.tile_pool(name="xt", bufs=2))
            yt_pool = ctx.enter_context(tc.tile_pool(name="yt", bufs=2))
            emb_pool = ctx.enter_context(tc.tile_pool(name="emb", bufs=2))
            qt_pool = ctx.enter_context(tc.tile_pool(name="qt", bufs=3))
            kt_pool = ctx.enter_context(tc.tile_pool(name="kt", bufs=3))
            v_pool = ctx.enter_context(tc.tile_pool(name="v", bufs=3))
            exp_pool = ctx.enter_context(tc.tile_pool(name="exp", bufs=3))
            expT_pool = ctx.enter_context(tc.tile_pool(name="expT", bufs=4))
            r2_pool = ctx.enter_context(tc.tile_pool(name="r2", bufs=3))
            attT_pool = ctx.enter_context(tc.tile_pool(name="attT", bufs=4))
            ystage_pool = ctx.enter_context(
                tc.tile_pool(name="ystage", bufs=2)
            )
            ps_qmk = ctx.enter_context(
                tc.tile_pool(name="ps_qmk", bufs=1, space="PSUM")
            )
            ps_v = ctx.enter_context(
                tc.tile_pool(name="ps_v", bufs=1, space="PSUM")
            )
            ps_qk = ctx.enter_context(
                tc.tile_pool(name="ps_qk", bufs=2, space="PSUM")
            )
            ps_av = ctx.enter_context(
                tc.tile_pool(name="ps_av", bufs=2, space="PSUM")
            )
            ps_y = ctx.enter_context(
                tc.tile_pool(name="ps_y", bufs=1, space="PSUM")
            )
            wq_sb = singles.tile([C, C], BF16)
            nc.sync.dma_start(out=wq_sb, in_=wq[:, :])
            wk_sb = singles.tile([C, C], BF16)
            nc.sync.dma_start(out=wk_sb, in_=wk[:, :])
            wv_sb = singles.tile([C, C], BF16)
            nc.sync.dma_start(out=wv_sb, in_=wv[:, :])
            pw_sb = singles.tile([C, C], BF16)
            nc.sync.dma_start(out=pw_sb, in_=pw[:, :])
            pb_sb = singles.tile([C, 1], F32)
            nc.sync.dma_start(out=pb_sb, in_=pb[:, :])
            ones_sb = singles.tile([N, HD], BF16)
            nc.vector.memset(ones_sb, 1.0)
            # persistent masked-q staging buffers: only the per-head
            # diagonal blocks are ever (re)written by the casts below,
            # off-head blocks stay zero from this one-time memset.
            qtbufs = []
            for i in range(3):
                qb = singles.tile([C, H, G4 * N], BF16, name=f"qtb{i}")
                nc.gpsimd.memset(qb, 0.0)
                qtbufs.append(qb)

            xt_ch = yt_ch = emb_ch = None
            qt_g = kt_g = psy = ystage = None

            for pair in range(n_pairs):
                w0 = 2 * pair
                if w0 % XCH == 0:
                    ch = w0 // XCH
                    xt_ch = xt_pool.tile([C, XCH * N], BF16)
                    nc.sync.dma_start(
                        out=xt_ch, in_=xT[:, ch * XCH * N : (ch + 1) * XCH * N]
                    )
                    yt_ch = yt_pool.tile([C, XCH * N], BF16)
                    nc.sync.dma_start(
                        out=yt_ch, in_=yT[:, ch * XCH * N : (ch + 1) * XCH * N]
                    )
                if pair % ECH == 0:
                    ek = pair // ECH
                    emb_ch = emb_pool.tile([N, ECH, HB], BF16)
                    nc.sync.dma_start(
                        out=emb_ch, in_=emb[:, ek * ECH : (ek + 1) * ECH, :]
                    )

                if w0 % G4 == 0:
                    # q / k projections for w0 .. w0+3 (one matmul each)
                    goff = (w0 % XCH) * N
                    pq = ps_qmk.tile([C, 512], F32, name="pq", tag="pq")
                    nc.tensor.matmul(
                        out=pq[:, 0 : G4 * N],
                        lhsT=wq_sb,
                        rhs=yt_ch[:, goff : goff + G4 * N],
                    )
                    qt_g = qtbufs[(w0 // G4) % 3]
                    for h in range(H):
                        nc.vector.tensor_copy(
                            qt_g[h * HD : (h + 1) * HD, h, :],
                            pq[h * HD : (h + 1) * HD, 0 : G4 * N],
                        )
                    pk = ps_qmk.tile([C, 512], F32, name="pk", tag="pk")
                    nc.tensor.matmul(
                        out=pk[:, 0 : G4 * N],
                        lhsT=wk_sb,
                        rhs=xt_ch[:, goff : goff + G4 * N],
                    )
                    kt_g = kt_pool.tile([C, G4 * N], BF16)
                    nc.vector.tensor_copy(kt_g, pk[:, 0 : G4 * N])

                # ---- v projections, batched 4 windows per psum tile/copy
                if w0 % G4 == 0:
                    pv4 = ps_v.tile([N, G4, 128], F32)
                    for j in range(G4):
                        col = ((w0 + j) % XCH) * N
                        nc.tensor.matmul(
                            out=pv4[:, j, 0:C],
                            lhsT=xt_ch[:, col : col + N],
                            rhs=wv_sb,
                        )
                    v4_sb = v_pool.tile([N, G4, C], BF16)
                    nc.scalar.copy(v4_sb, pv4[:, :, 0:C])

                # ---- qk logits + exp per window
                exp_pair = exp_pool.tile([N, 2, HB], BF16)
                for k in range(2):
                    w = w0 + k
                    i4 = (w % G4) * N
                    pqk = ps_qk.tile([N, 512], F32)
                    nc.tensor.matmul(
                        out=pqk[:, 0:HB],
                        lhsT=kt_g[:, i4 : i4 + N],
                        rhs=qt_g[:, :, i4 : i4 + N],
                    )
                    nc.scalar.activation(
                        out=exp_pair[:, k, :],
                        in_=pqk[:, 0:HB],
                        func=mybir.ActivationFunctionType.Exp,
                    )

                # ---- one multiply by emb for the pair (same distinct window)
                pj = pair % ECH
                expT = expT_pool.tile([N, 2, HB], BF16)
                emb_b = emb_ch[:, pj : pj + 1, :].broadcast_to((N, 2, HB))
                nc.vector.tensor_tensor(
                    out=expT, in0=exp_pair, in1=emb_b, op=mybir.AluOpType.mult
                )

                # ---- denominators: one [C, 4N] psum tile per 4 windows
                if pair % 2 == 0:
                    pdbc = ps_av.tile([C, 512], F32, name="pdbc", tag="avdbc")
                doff = (pair % 2) * 2 * N
                for h in range(H):
                    nc.tensor.matmul(
                        out=pdbc[h * HD : (h + 1) * HD, doff : doff + 2 * N],
                        lhsT=ones_sb,
                        rhs=expT[:, :, h * N : (h + 1) * N],
                    )
                if pair % 2 == 1:
                    # 1/d = exp(-ln(d)); Ln+Exp share one ACT table set
                    t_ln = r2_pool.tile([C, 4 * N], F32, name="t_ln", tag="tl")
                    nc.scalar.activation(
                        out=t_ln,
                        in_=pdbc[:, 0 : 4 * N],
                        func=mybir.ActivationFunctionType.Ln,
                    )
                    r2 = r2_pool.tile([C, 4 * N], F32, name="r2", tag="r2")
                    nc.scalar.activation(
                        out=r2,
                        in_=t_ln,
                        func=mybir.ActivationFunctionType.Exp,
                        scale=-1.0,
                    )
                    # ---- av + norm for the 4 windows, then one batched proj
                    g0 = w0 - 2
                    psy = ps_y.tile([C, 512], F32)
                    attT4 = attT_pool.tile([C, G4 * N], BF16)
                    for kk in range(2):
                        ep = expT_prev if kk == 0 else expT
                        pav = ps_av.tile([C, 512], F32, name="pav", tag="avdbc")
                        for k in range(2):
                            j = 2 * kk + k
                            for h in range(H):
                                nc.tensor.matmul(
                                    out=pav[
                                        h * HD : (h + 1) * HD,
                                        k * N : (k + 1) * N,
                                    ],
                                    lhsT=v4_sb[:, j, h * HD : (h + 1) * HD],
                                    rhs=ep[:, k, h * N : (h + 1) * N],
                                )
                        nc.vector.tensor_tensor(
                            out=attT4[:, kk * 2 * N : (kk + 1) * 2 * N],
                            in0=pav[:, 0 : 2 * N],
                            in1=r2[:, kk * 2 * N : (kk + 1) * 2 * N],
                            op=mybir.AluOpType.mult,
                        )
                    nc.tensor.matmul(
                        out=psy[:, 0 : G4 * N],
                        lhsT=pw_sb,
                        rhs=attT4,
                    )
                    # bias add during PSUM->SBUF staging, then DMA out per 8
                    if (g0 // G4) % 2 == 0:
                        ystage = ystage_pool.tile([C, YB * N], F32)
                    yoff = ((g0 // G4) % 2) * G4 * N
                    nc.scalar.activation(
                        out=ystage[:, yoff : yoff + G4 * N],
                        in_=psy[:, 0 : G4 * N],
                        func=mybir.ActivationFunctionType.Identity,
                        bias=pb_sb,
                    )
                    if (g0 + G4) % YB == 0:
                        blk = g0 // YB
                        nc.sync.dma_start(
                            out=out[:, blk * YB * N : (blk + 1) * YB * N],
                            in_=ystage,
                        )
                expT_prev = expT
    if split_waits:
        _split_sync_waits(nc)
    return nc


def _get_program():
    global _PROGRAM
    if _PROGRAM is None:
        _PROGRAM = _build_program()
    return _PROGRAM


# ------------------------------------------------------------------- kernel
def _core_instance_bidx(c):
    """B_ indices for core c's 128 window-instances, in device order."""
    w = np.arange(NI)
    return 512 * (w % 2) + NJ * c + (w // 2)


def _prepare_in_maps(x, y, mask, qkv_w, rpb_table, proj_w, proj_b):
    x = np.asarray(x, dtype=np.float32)
    y = np.asarray(y, dtype=np.float32)
    mask = np.asarray(mask, dtype=np.float32)
    qkv_w = np.asarray(qkv_w, dtype=np.float32)
    rpb_table = np.asarray(rpb_table, dtype=np.float32)
    proj_w = np.asarray(proj_w, dtype=np.float32)
    proj_b = np.asarray(proj_b, dtype=np.float32)

    scale = float(HD) ** -0.5

    # emb[wg, h, tq, tk] = exp(mask[wg, tq, tk] + bias[h, tq, tk])
    bias = rpb_table[REL_IDX.reshape(-1)].reshape(N, N, H).transpose(2, 0, 1)
    emb_all = np.exp(mask[:, None, :, :] + bias[None, :, :, :])
    # device layout [tk, wg, h*98+tq]
    emb_t = np.ascontiguousarray(emb_all.transpose(3, 0, 1, 2)).reshape(
        N, NW, HB
    )

    wq_h = np.ascontiguousarray((scale * qkv_w[0:C]).T).astype(NPBF16)
    wk_h = np.ascontiguousarray(qkv_w[C : 2 * C].T).astype(NPBF16)
    wv_h = np.ascontiguousarray(qkv_w[2 * C : 3 * C].T).astype(NPBF16)
    pw_h = np.ascontiguousarray(proj_w.T).astype(NPBF16)
    pb_h = np.ascontiguousarray(proj_b.reshape(C, 1)).astype(np.float32)

    in_maps = []
    bidx = []
    for c in range(NCORES):
        bi = _core_instance_bidx(c)
        bidx.append(bi)
        xc = x[bi].reshape(T, C)
        yc = y[bi].reshape(T, C)
        emb_c = np.ascontiguousarray(
            emb_t[:, NJ * c : NJ * (c + 1), :]
        ).astype(NPBF16)
        in_maps.append(
            {
                "xT": np.ascontiguousarray(xc.T).astype(NPBF16),
                "yT": np.ascontiguousarray(yc.T).astype(NPBF16),
                "emb": emb_c,
                "wq": wq_h,
                "wk": wk_h,
                "wv": wv_h,
                "pw": pw_h,
                "pb": pb_h,
            }
        )
    return in_maps, bidx


def kernel(x, y, mask, qkv_w, rpb_table, proj_w, proj_b):
    in_maps, bidx = _prepare_in_maps(
        x, y, mask, qkv_w, rpb_table, proj_w, proj_b
    )
    nc = _get_program()
    res = run_bass_kernel_spmd(nc, in_maps, list(range(NCORES)))

    out_full = np.empty((BWIN, N, C), dtype=np.float32)
    for c in range(NCORES):
        yt_o = np.asarray(res.results[c]["yT_out"], dtype=np.float32)
        out_full[bidx[c]] = yt_o.T.reshape(NI, N, C)
    return out_full

